# revision 23
# baseline (speedup 1.0000x reference)
"""ButterflyLinear Trainium2 kernel (fp8 I/O, identity-correction form).

Math insight: every one of the 12 butterfly stages pairs features strictly
within aligned groups of 4 (stage 0 pairs (4k,4k+1),(4k+2,4k+3); stages 1..11
all pair (4k,4k+2),(4k+1,4k+3)).  The whole network therefore collapses
exactly to a block-diagonal linear map with 1024 independent 4x4 blocks:

    out[t, 4k+j] = sum_i x[t, 4k+i] * M_k[i, j] + bias[4k+j]

M is extracted on the host (float64) by pushing the 4 group-basis vectors
through the stage chain.  The factors are identity + 0.01 noise, so
M = I + E with |E| <~ 0.15.  The device computes only the small correction

    c = E^T x                   (|c| <~ 0.65)

in fp8e4m3 end to end (x, E and c all fp8; fp32 PSUM accumulation), and the
host forms out = x_fp32 + c + bias, which restores full precision on the
dominant identity term.  Measured rel err ~1.0e-2 against the fp32
reference (gate 2e-2).  fp8 quarters HBM traffic vs fp32 (~8.9MB/core at
the ~425GB/s per-core DMA cap) and keeps matmuls single-pass on the PE.

Device pipeline: the host ships x pre-transposed in fp8 (feature-major
group tiles, 8KB-contiguous rows).  Each 128-feature chunk runs two
stationary-weight matmuls (512 moving tokens each) into a 2-bank fp32
PSUM tile (4 tiles rotating = all 8 banks, so the PE runs ahead of the
copies); one per-chunk PSUM->SBUF copy downcasts to fp8.  The copies
alternate between the ACT and DVE engines (ACT is ~9% faster per copy
and takes one extra chunk) and are the steady-state pacer at ~0.64us
per chunk across both engines.  Loads AND stores share the sync DMA
queue, ordered loads-first, so the input stream is never starved by
store traffic while compute still needs data; the queue order releases
a store only after the loads compute depends on, and the tail is one
4-chunk store since the drain is copy-paced, not store-latency-paced.
E rides the ACT queue so it never serializes ahead of the x stream.

Sharding: data-parallel over tokens, 8192/8 = 1024 tokens per core.
"""

import numpy as np

TOKENS = 8192
N = 4096
DEPTH = 12
NCORES = 8
TOK_PER_CORE = TOKENS // NCORES  # 1024
P = 128                  # partitions
N_CHUNKS = N // P        # 32 feature chunks of 128
GRP = 8                  # chunks per group tile (8*1024 tok*1B = 8KB rows)
N_GROUPS = N_CHUNKS // GRP     # 4
TBLK = 512               # moving-token block per matmul (one PSUM bank fp32)
N_TBLK = TOK_PER_CORE // TBLK  # 2


def _apply_stage_np(x, factor, stage):
    B, n = x.shape
    block = 1 << (stage + 1)
    half = block >> 1
    m = n // block
    staged = x.reshape(B, m, half, 2).transpose(0, 1, 3, 2)
    pairs = staged.reshape(B, n // 2, 2)
    t = np.einsum("bnc,ncd->bnd", pairs, factor)
    t = t.reshape(B, m, 2, half).transpose(0, 1, 3, 2)
    return t.reshape(B, n)


def _compose_weights(factors):
    """Return W [128, N] float64: W[k, c*128+m] = weight(in k, out m) of
    chunk c, i.e. Mblock[k%4, m%4] of group (c*128+m)//4 when k//4==m//4,
    else 0."""
    V = np.zeros((4, N), dtype=np.float64)
    for i in range(4):
        V[i, i::4] = 1.0
    M = V
    f64 = np.asarray(factors, dtype=np.float64)
    for s in range(DEPTH):
        M = _apply_stage_np(M, f64[s], s)
    # M[i, col] = Mfull[4*(col//4)+i, col]
    kk = np.arange(P)
    cols = np.arange(N)
    W = M[kk % 4][:, :]                     # [128, N]
    mask = ((cols[None, :] % P) // 4) == (kk[:, None] // 4)
    return W * mask


_PROG = None


def _get_program():
    global _PROG
    if _PROG is not None:
        return _PROG

    import concourse.mybir as mybir
    import concourse.tile as tile
    from concourse import bacc

    nc = bacc.Bacc("TRN2", target_bir_lowering=False, debug=False,
                   num_devices=NCORES)
    f32 = mybir.dt.float32
    f8 = mybir.dt.float8e4
    xp_h = nc.dram_tensor("xp", [N_GROUPS, P, GRP, TOK_PER_CORE], f8,
                          kind="ExternalInput")
    w_h = nc.dram_tensor("w", [P, N], f8, kind="ExternalInput")
    op_h = nc.dram_tensor("outp", [N_GROUPS, P, GRP, TOK_PER_CORE], f8,
                          kind="ExternalOutput")

    xp = xp_h.ap()
    op = op_h.ap()

    # Units (start chunk, n chunks): small first/last units prime and
    # drain the pipeline fast; big middle units keep 8KB contiguous
    # per-partition DMA rows.
    UNITS = [(0, 2), (2, 2), (4, 4), (8, 8), (16, 8), (24, 4),
             (28, 4)]
    # Single-queue program order: all loads compute needs soon go first,
    # each store is released only after later loads are already enqueued.
    # The tail is one 4-chunk store (one issue, 4KB rows) since the drain
    # is paced by the copy engines, not store latency.
    ORDER = ["L0", "L1", "L2", "L3", "L4", "L5", "S0", "L6", "S1",
             "S2", "S3", "S4", "S5", "S6"]

    with tile.TileContext(nc) as tc:
        with (
            tc.tile_pool(name="singles", bufs=1) as singles,
            tc.tile_pool(name="xin", bufs=6) as xpool,
            tc.tile_pool(name="oout", bufs=5) as opool,
            tc.tile_pool(name="ps", bufs=4, space="PSUM") as pspool,
        ):
            # E rides the otherwise-idle ACT DMA queue so it never
            # serializes ahead of the x stream; the first 4 chunks load
            # separately so chunk 0's matmul isn't gated on the rest.
            w_sb = singles.tile([P, N], f8)
            nc.scalar.dma_start(out=w_sb[:, 0:4 * P],
                                in_=w_h.ap()[:, 0:4 * P])
            nc.scalar.dma_start(out=w_sb[:, 4 * P:],
                                in_=w_h.ap()[:, 4 * P:])

            xgs = {}
            ogs = {}
            for tok in ORDER:
                u = int(tok[1:])
                c0, nch = UNITS[u]
                g, base = c0 // GRP, c0 % GRP
                if tok[0] == "L":
                    xg = xpool.tile([P, GRP, TOK_PER_CORE], f8, tag="xg")
                    xgs[u] = xg
                    nc.sync.dma_start(
                        out=xg[:, 0:nch, :],
                        in_=xp[g, :, base:base + nch, :])
                    # Compute for this unit, interleaved right after its
                    # load is enqueued (engines proceed on data deps).
                    og = opool.tile([P, GRP, TOK_PER_CORE], f8, tag="og")
                    ogs[u] = og
                    for cc in range(nch):
                        c = c0 + cc
                        ps = pspool.tile([P, TOK_PER_CORE], f32)  # 2 banks
                        for tb in range(N_TBLK):
                            nc.tensor.matmul(
                                ps[:, tb * TBLK:(tb + 1) * TBLK],
                                lhsT=w_sb[:, c * P:(c + 1) * P],
                                rhs=xg[:, cc, tb * TBLK:(tb + 1) * TBLK],
                                start=True, stop=True,
                            )
                            if c in (0, N_CHUNKS - 1):
                                # Stream head/tail: halve copy latency by
                                # splitting the chunk across both engines,
                                # each half right behind its own matmul.
                                half = ps[:, tb * TBLK:(tb + 1) * TBLK]
                                dsth = og[:, cc, tb * TBLK:(tb + 1) * TBLK]
                                if tb == 0:
                                    nc.scalar.copy(dsth, half)
                                else:
                                    nc.vector.tensor_scalar_add(
                                        dsth, half, 0.0)
                        if c in (0, N_CHUNKS - 1):
                            continue
                        dst = og[:, cc, :]
                        # ACT is ~9% faster per copy than DVE; give it the
                        # even chunks plus one extra mid-stream (17:15).
                        if c % 2 == 0 or c == 15:
                            nc.scalar.copy(dst, ps)
                        else:
                            nc.vector.tensor_scalar_add(dst, ps, 0.0)
                else:
                    nc.sync.dma_start(
                        out=op[g, :, base:base + nch, :],
                        in_=ogs[u][:, 0:nch, :])

    nc.compile()
    _PROG = nc
    return nc


def _prep_core_input(xs8):
    """[1024, 4096] fp8 token-major -> [4, 128, 8, 1024] feature-major tiles.

    xprep[g, p, cc, t] = xs[t, (8g+cc)*128 + p]
    """
    xt = xs8.T.reshape(N_GROUPS, GRP, P, TOK_PER_CORE)   # [g][cc][p][t]
    return np.ascontiguousarray(xt.transpose(0, 2, 1, 3))


def _unprep_core_output(outp):
    """Inverse of _prep_core_input (fp8 -> fp32 [1024, 4096] token-major)."""
    o = np.asarray(outp).transpose(0, 2, 1, 3)           # [g][cc][p][t]
    return o.reshape(N, TOK_PER_CORE).T.astype(np.float32)


def kernel(x, factors, bias):
    import ml_dtypes
    from concourse.bass_utils import run_bass_kernel_spmd

    f8np = ml_dtypes.float8_e4m3

    x = np.asarray(x, dtype=np.float32)
    factors = np.asarray(factors, dtype=np.float32)
    bias_np = np.asarray(bias, dtype=np.float32)
    assert x.shape == (TOKENS, N)

    x8 = x.astype(f8np)
    W = _compose_weights(factors)
    E = W.copy()
    for c in range(N_CHUNKS):
        blk = E[:, c * P:(c + 1) * P]
        blk[np.arange(P), np.arange(P)] -= 1.0
    w8 = np.ascontiguousarray(E.astype(f8np))

    nc = _get_program()
    in_maps = []
    for c in range(NCORES):
        in_maps.append({
            "xp": _prep_core_input(x8[c * TOK_PER_CORE:(c + 1) * TOK_PER_CORE]),
            "w": w8,
        })
    res = run_bass_kernel_spmd(nc, in_maps, core_ids=list(range(NCORES)))
    out = np.empty((TOKENS, N), dtype=np.float32)
    for c in range(NCORES):
        out[c * TOK_PER_CORE:(c + 1) * TOK_PER_CORE] = (
            x[c * TOK_PER_CORE:(c + 1) * TOK_PER_CORE]
            + _unprep_core_output(res.results[c]["outp"]))
    out += bias_np[None, :]
    return out
